# revision 1
# baseline (speedup 1.0000x reference)
"""LocalTrittention TRN2 kernel: 8-core batch-data-parallel Bass/Tile implementation.

Problem (B=64, S=256, HID=4096, H=16, D=256, WINDOW=64):
  q,k1,k2,v1,v2 = hs @ W*.T + b*            (5 projections, per-head split)
  s1 = q @ k1^T ; scores = (s1 @ k2^T) * 1/sqrt(D)   (per (b,h), S==D)
  scores[:, S-WINDOW:] = -inf ; probs = softmax(scores)
  out = probs @ (v1+v2)  -> [B,S,HID]

Sharding: batch (64) split across 8 cores (8 batches/core). Weights replicated.
Host prep: layout only (transpose hs shard and the 5 weight matrices so the
contraction index is partition-major); all FLOPs run on device.

Device math runs in fp32r (fp32 rounded to 11 mantissa bits, 4x faster
matmul); accumulation is fp32 in PSUM; softmax in fp32.
"""

import sys, time

sys.path.insert(0, "/opt/trn_rl_repo")

import numpy as np

import concourse.bass as bass
import concourse.tile as tile
from concourse import bacc, mybir
from concourse.masks import make_identity

B, S, HID = 64, 256, 4096
H, D = 16, 256
WINDOW = 64
SV = S - WINDOW  # valid (unmasked) score columns
SCALE = 1.0 / float(np.sqrt(D))

NCORES = 8
BPC = B // NCORES  # batches per core
T = BPC * S  # tokens per core (2048)
KC = HID // 128  # contraction chunks (32)
HALF = T // 2  # token half (1024)

F32 = mybir.dt.float32
F32R = mybir.dt.float32r
AX = mybir.AxisListType.X
EXP = mybir.ActivationFunctionType.Exp


def build_bass(reps=1):
    nc = bacc.Bacc("TRN2", target_bir_lowering=False, debug=True)

    hsT = nc.dram_tensor("hsT", [HID, T], F32, kind="ExternalInput")
    wts = {
        n: nc.dram_tensor(f"w{n}T", [HID, HID], F32, kind="ExternalInput")
        for n in ("q", "k1", "k2", "v1", "v2")
    }
    bqs = {
        n: nc.dram_tensor(f"b{n}", [HID], F32, kind="ExternalInput")
        for n in ("q", "k1", "k2", "v1", "v2")
    }
    outd = nc.dram_tensor("out", [T, HID], F32, kind="ExternalOutput")

    with tile.TileContext(nc) as tc:
        with (
            tc.tile_pool(name="const", bufs=1) as const,
            tc.tile_pool(name="dram", bufs=1, space="DRAM") as dram,
        ):
            ident = const.tile([128, 128], F32)
            make_identity(nc, ident[:])

            # per-partition bias tiles [128, 32] (o-chunk-major) for q/k1/k2
            bias_sb = {}
            for n in ("q", "k1", "k2"):
                t = const.tile([128, KC], F32, name=f"bias_{n}")
                nc.sync.dma_start(t[:], bqs[n].ap().rearrange("(m p) -> p m", p=128))
                bias_sb[n] = t
            # broadcast bias (bv1+bv2) [128, HID] for the ctx epilogue
            bv1b = const.tile([128, HID], F32)
            nc.sync.dma_start(bv1b[:], bqs["v1"].ap().partition_broadcast(128))
            bv2b = const.tile([128, HID], F32)
            nc.sync.dma_start(bv2b[:], bqs["v2"].ap().partition_broadcast(128))
            biasb = const.tile([128, HID], F32)
            nc.vector.tensor_add(biasb[:], bv1b[:], bv2b[:])

            # intermediate DRAM (fp32r): qT/k1T/k2T [HID, T] d-major, v [T, HID]
            qTd = dram.tile([HID, T], F32R, name="qTd")
            k1Td = dram.tile([HID, T], F32R, name="k1Td")
            k2Td = dram.tile([HID, T], F32R, name="k2Td")
            vd = dram.tile([T, HID], F32R, name="vd")
            projd = {"q": qTd, "k1": k1Td, "k2": k2Td}

            for _rep in range(reps):
                # ---------------- Phase A: projections ----------------
                with (
                    tc.tile_pool(name="hst", bufs=1) as hstp,
                    tc.tile_pool(name="wtile", bufs=6) as wtp,
                    tc.tile_pool(name="evac", bufs=6) as evp,
                    tc.tile_pool(name="apsum", bufs=8, space="PSUM") as apsum,
                ):
                    for hf in range(2):
                        cols = slice(hf * HALF, (hf + 1) * HALF)
                        hst = hstp.tile([128, KC, HALF], F32R, tag="hst", name="hst")
                        for k in range(KC):
                            nc.gpsimd.dma_start(
                                hst[:, k, :], hsT.ap()[k * 128 : (k + 1) * 128, cols]
                            )

                        # q/k1/k2: out-chunk-stationary (W tile), hsT moving
                        for n in ("q", "k1", "k2"):
                            wt = wts[n]
                            for mg in range(8):
                                pss = [
                                    apsum.tile(
                                        [128, 512], F32, tag="ps", name=f"ps{i}"
                                    )
                                    for i in range(8)
                                ]
                                for k in range(KC):
                                    wtile = wtp.tile(
                                        [128, 512], F32R, tag="wt", name="wtile"
                                    )
                                    nc.gpsimd.dma_start(
                                        wtile[:],
                                        wt.ap()[
                                            k * 128 : (k + 1) * 128,
                                            mg * 512 : (mg + 1) * 512,
                                        ],
                                    )
                                    for m in range(4):
                                        for nn in range(2):
                                            nc.tensor.matmul(
                                                pss[m * 2 + nn][:],
                                                wtile[:, m * 128 : (m + 1) * 128],
                                                hst[:, k, nn * 512 : (nn + 1) * 512],
                                                start=(k == 0),
                                                stop=(k == KC - 1),
                                            )
                                for m in range(4):
                                    for nn in range(2):
                                        ev = evp.tile(
                                            [128, 512], F32R, tag="ev", name="ev"
                                        )
                                        nc.vector.tensor_scalar_add(
                                            ev[:],
                                            pss[m * 2 + nn][:],
                                            bias_sb[n][:, mg * 4 + m : mg * 4 + m + 1],
                                        )
                                        nc.sync.dma_start(
                                            projd[n][
                                                mg * 512 + m * 128 : mg * 512 + (m + 1) * 128,
                                                hf * HALF + nn * 512 : hf * HALF + (nn + 1) * 512,
                                            ],
                                            ev[:],
                                        )

                        # v = hs@(wv1.T) + hs@(wv2.T): hsT stationary, W moving
                        for ng in range(8):
                            pss = [
                                apsum.tile([128, 512], F32, tag="ps", name=f"vps{i}")
                                for i in range(8)
                            ]
                            for k in range(KC):
                                wv_tiles = []
                                for wi, n in enumerate(("v1", "v2")):
                                    wtile = wtp.tile(
                                        [128, 512], F32R, tag="wt", name="wvtile"
                                    )
                                    nc.gpsimd.dma_start(
                                        wtile[:],
                                        wts[n].ap()[
                                            k * 128 : (k + 1) * 128,
                                            ng * 512 : (ng + 1) * 512,
                                        ],
                                    )
                                    wv_tiles.append(wtile)
                                for wi in range(2):
                                    for m in range(8):
                                        nc.tensor.matmul(
                                            pss[m][:],
                                            hst[:, k, m * 128 : (m + 1) * 128],
                                            wv_tiles[wi][:],
                                            start=(k == 0 and wi == 0),
                                            stop=(k == KC - 1 and wi == 1),
                                        )
                            for m in range(8):
                                ev = evp.tile([128, 512], F32R, tag="ev", name="vev")
                                nc.vector.tensor_copy(ev[:], pss[m][:])
                                nc.sync.dma_start(
                                    vd[
                                        hf * HALF + m * 128 : hf * HALF + (m + 1) * 128,
                                        ng * 512 : (ng + 1) * 512,
                                    ],
                                    ev[:],
                                )

                # ---------------- Phase B: attention ----------------
                with (
                    tc.tile_pool(name="bio", bufs=3) as bio,
                    tc.tile_pool(name="bwork", bufs=2) as bw,
                    tc.tile_pool(name="bps", bufs=2, space="PSUM") as bps,
                ):
                    for b in range(BPC):
                        for h in range(H):
                            rows = slice(h * S, (h + 1) * S)
                            colsb = slice(b * S, (b + 1) * S)
                            qt = bio.tile([128, 2, S], F32R, tag="qt", name="qt")
                            k1 = bio.tile([128, 2, S], F32R, tag="k1", name="k1")
                            k2 = bio.tile([128, 2, S], F32R, tag="k2", name="k2")
                            vt = bio.tile([128, 2, S], F32R, tag="vt", name="vt")
                            for t_, d_ in ((qt, qTd), (k1, k1Td), (k2, k2Td)):
                                nc.sync.dma_start(
                                    t_[:],
                                    d_[rows, colsb].rearrange("(c p) s -> p c s", p=128),
                                )
                            nc.sync.dma_start(
                                vt[:],
                                vd[colsb, rows].rearrange("(c p) s -> p c s", p=128),
                            )

                            # s1T[m,q] = sum_d k1T[d,m] qT[d,q], scaled
                            s1r = bw.tile([128, 2, S], F32R, tag="s1r", name="s1r")
                            for m in range(2):
                                ps = bps.tile([128, S], F32, tag="s1ps", name="s1ps")
                                for d_ in range(2):
                                    nc.tensor.matmul(
                                        ps[:],
                                        k1[:, d_, bass.ts(m, 128)],
                                        qt[:, d_, :],
                                        start=(d_ == 0),
                                        stop=(d_ == 1),
                                    )
                                nc.vector.tensor_scalar_mul(s1r[:, m, :], ps[:], SCALE)

                            # scores[q,j] (full N=S), softmax over j<SV
                            probs = bw.tile([128, 2, SV], F32, tag="probs", name="probs")
                            recip = bw.tile([128, 2], F32, tag="recip", name="recip")
                            for q in range(2):
                                ps = bps.tile([128, S], F32, tag="scps", name="scps")
                                for m in range(2):
                                    nc.tensor.matmul(
                                        ps[:],
                                        s1r[:, m, bass.ts(q, 128)],
                                        k2[:, m, :],
                                        start=(m == 0),
                                        stop=(m == 1),
                                    )
                                negmax = bw.tile([128, 1], F32, tag="ngm", name="ngm")
                                nc.vector.reduce_max(
                                    negmax[:], ps[:, :SV], axis=AX, negate=True
                                )
                                sumexp = bw.tile([128, 1], F32, tag="sme", name="sme")
                                nc.scalar.activation(
                                    probs[:, q, :],
                                    ps[:, :SV],
                                    EXP,
                                    bias=negmax[:],
                                    scale=1.0,
                                    accum_out=sumexp[:],
                                )
                                nc.vector.reciprocal(recip[:, q : q + 1], sumexp[:])

                            # transpose probs (valid cols only) -> fp32r
                            ptr = bw.tile([128, 2, S], F32R, tag="ptr", name="ptr")
                            for q in range(2):
                                pst = bps.tile([128, S], F32, tag="pst", name="pst")
                                nc.tensor.transpose(
                                    pst[:, bass.ts(0, 128)], probs[:, q, :128], ident[:]
                                )
                                nc.tensor.transpose(
                                    pst[:64, bass.ds(128, 128)],
                                    probs[:, q, 128:SV],
                                    ident[:],
                                )
                                nc.vector.tensor_copy(ptr[:, q, :], pst[:])

                            # ctx[q,d] = sum_{j<SV} probsT[j,q] v[j,d]; normalize+bias
                            ctxs = bw.tile([128, 2, S], F32, tag="ctxs", name="ctxs")
                            for q in range(2):
                                ps = bps.tile([128, S], F32, tag="ctxps", name="ctxps")
                                nc.tensor.matmul(
                                    ps[:],
                                    ptr[:, q, :128],
                                    vt[:, 0, :],
                                    start=True,
                                    stop=False,
                                )
                                nc.tensor.matmul(
                                    ps[:],
                                    ptr[:64, q, 128:256],
                                    vt[:64, 1, :],
                                    start=False,
                                    stop=True,
                                )
                                nc.vector.tensor_scalar_mul(
                                    ctxs[:, q, :], ps[:], recip[:, q : q + 1]
                                )
                                nc.vector.tensor_add(
                                    ctxs[:, q, :], ctxs[:, q, :], biasb[:, rows]
                                )

                            nc.sync.dma_start(
                                outd.ap()[colsb, rows].rearrange(
                                    "(c p) s -> p c s", p=128
                                ),
                                ctxs[:],
                            )
    nc.compile()
    return nc


# ---------------------------------------------------------------------------
# host-side runner (mirrors bass2jax.run_bass_via_pjrt with device-resident
# inputs; weights replicated across cores rather than concatenated)
# ---------------------------------------------------------------------------

_CACHE = {}


def _run(nc, in_maps, n_cores, replicated=(), time_reps=0):
    import jax
    from jax.sharding import Mesh, PartitionSpec, NamedSharding
    from jax.experimental.shard_map import shard_map
    from concourse.bass2jax import (
        install_neuronx_cc_hook,
        _bass_exec_p,
        partition_id_tensor,
    )

    install_neuronx_cc_hook()

    if nc.dbg_addr is not None:
        assert not nc.dbg_callbacks
        in_maps = [
            {**m, nc.dbg_addr.name: np.zeros((1, 2), np.uint32)} for m in in_maps
        ]

    partition_name = nc.partition_id_tensor.name if nc.partition_id_tensor else None

    in_names, out_names, out_avals, zero_outs = [], [], [], []
    for alloc in nc.m.functions[0].allocations:
        if not isinstance(alloc, mybir.MemoryLocationSet):
            continue
        name = alloc.memorylocations[0].name
        if alloc.kind == "ExternalInput":
            if name != partition_name:
                in_names.append(name)
        elif alloc.kind == "ExternalOutput":
            out_names.append(name)
            shape = tuple(alloc.tensor_shape)
            dtype = mybir.dt.np(alloc.dtype)
            out_avals.append(jax.core.ShapedArray(shape, dtype))
            zero_outs.append(np.zeros(shape, dtype))
    n_params = len(in_names)
    n_outs = len(out_avals)
    param_names = list(in_names)
    in_names = in_names + out_names
    if partition_name is not None:
        in_names.append(partition_name)

    donate = tuple(range(n_params, n_params + n_outs))

    def _body(*args):
        operands = list(args)
        if partition_name is not None:
            operands.append(partition_id_tensor())
        outs = _bass_exec_p.bind(
            *operands,
            out_avals=tuple(out_avals),
            in_names=tuple(in_names),
            out_names=tuple(out_names),
            lowering_input_output_aliases=(),
            sim_require_finite=True,
            sim_require_nnan=True,
            nc=nc,
        )
        return tuple(outs)

    devices = jax.devices()[:n_cores]
    mesh = Mesh(np.asarray(devices), ("core",))
    rep = set(replicated)
    in_specs = tuple(
        PartitionSpec() if nm in rep else PartitionSpec("core")
        for nm in param_names
    ) + (PartitionSpec("core"),) * n_outs
    out_specs = (PartitionSpec("core"),) * len(out_names)
    sharded = jax.jit(
        shard_map(
            _body, mesh=mesh, in_specs=in_specs, out_specs=out_specs, check_rep=False
        ),
        donate_argnums=donate,
        keep_unused=True,
    )

    shard_sh = NamedSharding(mesh, PartitionSpec("core"))
    rep_sh = NamedSharding(mesh, PartitionSpec())
    concat_in = []
    for i, nm in enumerate(param_names):
        if nm in rep:
            concat_in.append(jax.device_put(np.asarray(in_maps[0][nm]), rep_sh))
        else:
            concat_in.append(
                jax.device_put(
                    np.concatenate(
                        [np.asarray(in_maps[c][nm]) for c in range(n_cores)], axis=0
                    ),
                    shard_sh,
                )
            )
    jax.block_until_ready(concat_in)

    def fresh_zeros():
        zs = [
            jax.device_put(np.zeros((n_cores * z.shape[0], *z.shape[1:]), z.dtype), shard_sh)
            for z in zero_outs
        ]
        jax.block_until_ready(zs)
        return zs

    t0 = time.perf_counter()
    out_arrs = jax.block_until_ready(sharded(*concat_in, *fresh_zeros()))
    first_call_s = time.perf_counter() - t0
    results = [
        {
            name: np.asarray(out_arrs[i]).reshape(n_cores, *out_avals[i].shape)[c]
            for i, name in enumerate(out_names)
        }
        for c in range(n_cores)
    ]

    # non-donating variant for timing bursts: zeros stay device-resident and
    # are reused across calls (the kernel writes every output element)
    sharded_nd = jax.jit(
        shard_map(
            _body, mesh=mesh, in_specs=in_specs, out_specs=out_specs, check_rep=False
        ),
        keep_unused=True,
    )
    zs_resident = fresh_zeros()

    def timed_burst(m):
        """Enqueue m executions back-to-back, fetch a few bytes of the last
        one's output. Device serializes the execs, so wall ~= dispatch
        overhead + m * exec_time once m*exec exceeds the RPC window."""
        t0 = time.perf_counter()
        outs = None
        for _ in range(m):
            outs = sharded_nd(*concat_in, *zs_resident)
        for o in outs:
            np.asarray(jax.device_get(o.addressable_shards[0].data[0:1, 0:8]))
        return time.perf_counter() - t0

    times = [timed_burst(1) for _ in range(time_reps)]

    return results, times, first_call_s, timed_burst


def kernel(
    hidden_states,
    wq,
    bq,
    wk1,
    bk1,
    wk2,
    bk2,
    wv1,
    bv1,
    wv2,
    bv2,
    _time_reps=0,
    _reps=1,
):
    hs = np.asarray(hidden_states, dtype=np.float32)
    weights = {
        "q": np.asarray(wq, np.float32),
        "k1": np.asarray(wk1, np.float32),
        "k2": np.asarray(wk2, np.float32),
        "v1": np.asarray(wv1, np.float32),
        "v2": np.asarray(wv2, np.float32),
    }
    biases = {
        "q": np.asarray(bq, np.float32),
        "k1": np.asarray(bk1, np.float32),
        "k2": np.asarray(bk2, np.float32),
        "v1": np.asarray(bv1, np.float32),
        "v2": np.asarray(bv2, np.float32),
    }

    if ("nc", _reps) not in _CACHE:
        _CACHE[("nc", _reps)] = build_bass(_reps)
    nc = _CACHE[("nc", _reps)]

    # host prep: layout only (transposes), no arithmetic
    wT = {n: np.ascontiguousarray(w.T) for n, w in weights.items()}
    in_maps = []
    for c in range(NCORES):
        shard = hs[c * BPC : (c + 1) * BPC].reshape(T, HID)
        m = {"hsT": np.ascontiguousarray(shard.T)}
        for n in ("q", "k1", "k2", "v1", "v2"):
            m[f"w{n}T"] = wT[n]
            m[f"b{n}"] = biases[n]
        in_maps.append(m)

    replicated = [f"w{n}T" for n in weights] + [f"b{n}" for n in biases]
    results, times, first_s, burst = _run(
        nc, in_maps, NCORES, replicated=replicated, time_reps=_time_reps
    )
    kernel._last_times = times
    kernel._first_call_s = first_s
    kernel._burst = burst

    out = np.empty((B, S, HID), np.float32)
    for c in range(NCORES):
        out[c * BPC : (c + 1) * BPC] = results[c]["out"].reshape(BPC, S, HID)
    return out

